# revision 7
# baseline (speedup 1.0000x reference)
"""AdaptiveBoundaryRankingLoss on 8 TRN2 NeuronCores.

Math: loss = sum_{i<j} relu(boundary(|dt|) - (p_i-p_j)*sign(dt)) / K,
  dt = t_i - t_j, boundary(a) = BETA*a/(1+GAMMA*a), K = B(B-1)/2.

Host sorts (pred,target) by target ascending (loss is a sum over
unordered pairs, so relabeling is free). After sorting, for i>j
(strict lower triangle) sign(t_i - t_j) = +1, so
  loss*K/BETA = sum_{i>j} relu(a*r - dp'),  a = t_i - t_j >= 0,
  r = 1/(1+GAMMA*a) = exp(-ln(1+GAMMA*a)),  dp' = (p_i - p_j)/BETA.

Engine split per 128-row tile (bf16 elementwise):
  ScalarE: L = Ln(GAMMA*(-tc) + (1+GAMMA*tr)); r = Exp(-L)
  VectorE: m = STT((-tc + tr) * r); z = STT((pc' - pr') + m) = m - dp';
           edge-mask via tensor_tensor min; w = relu(z) via tensor_scalar
  TensorE: psum[1,512] += ones[128,1]^T @ w[:,chunk]  (column-sum reduce)
Host: loss = BETA * sum(psum) / K.

Work split: 64 row-blocks of 128 rows; core c takes blocks {8k+c},
tile k spans columns [0,(k+1)*1024) -> identical graph on all cores
(SPMD); per-core differences live in input data (tables + edge mask).
"""

import contextlib

import numpy as np
import ml_dtypes

import concourse.bass as bass
from concourse import mybir
from concourse.bass_utils import run_bass_kernel_spmd

B = 8192
BETA = 0.3
GAMMA = 0.1
NCORES = 8
NT = 8          # tiles per core
TW = 1024       # column granularity; tile k has W_k = (k+1)*TW columns
P = 128
MMW = 512       # matmul chunk width (one PSUM bank of f32)

_bf16 = ml_dtypes.bfloat16

_NC_CACHE = None


def build_nc():
    nc = bass.Bass(target_bir_lowering=False, debug=False)
    f32 = mybir.dt.float32
    bf16 = mybir.dt.bfloat16
    A = mybir.AluOpType

    tcneg_d = nc.declare_dram_parameter("tcneg", [P, B], bf16, isOutput=False)
    pcb2_d = nc.declare_dram_parameter("pcb2", [P, B], bf16, isOutput=False)
    mask_d = nc.declare_dram_parameter("mask", [P, TW], bf16, isOutput=False)
    trt_d = nc.declare_dram_parameter("trt", [P, NT], f32, isOutput=False)
    lnb_d = nc.declare_dram_parameter("lnb", [P, NT], f32, isOutput=False)
    prb_d = nc.declare_dram_parameter("prb", [P, NT], f32, isOutput=False)
    ones_d = nc.declare_dram_parameter("ones", [P, 1], bf16, isOutput=False)
    out_d = nc.declare_dram_parameter("out", [1, MMW], f32, isOutput=True)

    NDMA = 7 * 16

    es = contextlib.ExitStack()
    with es:
        def sb(name, shape, dtype):
            return es.enter_context(nc.sbuf_tensor(name, shape, dtype))

        tcneg = sb("tcneg_s", [P, B], bf16)
        pcb2 = sb("pcb2_s", [P, B], bf16)
        mask = sb("mask_s", [P, TW], bf16)
        trt = sb("trt_s", [P, NT], f32)
        lnb = sb("lnb_s", [P, NT], f32)
        prb = sb("prb_s", [P, NT], f32)
        ones = sb("ones_s", [P, 1], bf16)
        Lb = sb("Lb", [P, B], bf16)
        r0 = sb("r0", [P, B], bf16)
        r1 = sb("r1", [P, B], bf16)
        mb = sb("mb", [P, B], bf16)
        zb = sb("zb", [P, B], bf16)
        w0 = sb("w0", [P, B], bf16)
        w1 = sb("w1", [P, B], bf16)
        osb = sb("osb", [1, MMW], f32)
        ps = es.enter_context(nc.psum_tensor("ps", [1, MMW], f32))
        dma_sem = es.enter_context(nc.semaphore("dma_sem"))
        se_sem = es.enter_context(nc.semaphore("se_sem"))
        ve_sem = es.enter_context(nc.semaphore("ve_sem"))
        te_sem = es.enter_context(nc.semaphore("te_sem"))
        block = es.enter_context(nc.Block())

        rbufs = [r0, r1]
        wbufs = [w0, w1]

        @block.sync
        def _(sync):
            for dst, src in [
                (tcneg, tcneg_d), (pcb2, pcb2_d), (mask, mask_d),
                (trt, trt_d), (lnb, lnb_d), (prb, prb_d), (ones, ones_d),
            ]:
                sync.dma_start(out=dst[:, :], in_=src[:, :]).then_inc(dma_sem, 16)
            sync.wait_ge(ve_sem, NT + 1)
            sync.dma_start(out=out_d[:, :], in_=osb[:, :]).then_inc(dma_sem, 16)

        @block.scalar
        def _(scalar):
            scalar.wait_ge(dma_sem, NDMA)
            for k in range(NT):
                W = (k + 1) * TW
                if k >= 2:
                    scalar.wait_ge(ve_sem, k - 1)
                # L = ln(1 + G*(tr - tc)) ; tcneg holds -tc
                scalar.activation(
                    Lb[:, :W], tcneg[:, :W],
                    mybir.ActivationFunctionType.Ln,
                    bias=lnb[:, k:k + 1], scale=GAMMA,
                )
                # r = exp(-L) = 1/(1 + G*a)
                scalar.activation(
                    rbufs[k % 2][:, :W], Lb[:, :W],
                    mybir.ActivationFunctionType.Exp,
                    scale=-1.0,
                ).then_inc(se_sem, 1)

        @block.vector
        def _(vector):
            vector.wait_ge(dma_sem, NDMA)
            for k in range(NT):
                W = (k + 1) * TW
                vector.wait_ge(se_sem, k + 1)
                # m = (tr - tc) * r
                vector.scalar_tensor_tensor(
                    out=mb[:, :W], in0=tcneg[:, :W], scalar=trt[:, k:k + 1],
                    in1=rbufs[k % 2][:, :W], op0=A.add, op1=A.mult,
                )
                # z = (pc' - pr') + m = m - dp'
                vector.scalar_tensor_tensor(
                    out=zb[:, :W], in0=pcb2[:, :W], scalar=prb[:, k:k + 1],
                    in1=mb[:, :W], op0=A.subtract, op1=A.add,
                )
                # edge window: invalid (j >= i) elements -> -30000
                vector.tensor_tensor(
                    out=zb[:, k * TW:W], in0=zb[:, k * TW:W], in1=mask[:, :],
                    op=A.min,
                )
                if k >= 2:
                    vector.wait_ge(te_sem, k - 1)
                # w = relu(z)
                vector.tensor_scalar(
                    out=wbufs[k % 2][:, :W], in0=zb[:, :W],
                    scalar1=0.0, scalar2=None, op0=A.max,
                ).then_inc(ve_sem, 1)
            vector.wait_ge(te_sem, NT)
            vector.tensor_copy(osb[:, :], ps[:, :]).then_inc(ve_sem, 1)

        @block.tensor
        def _(tensor):
            tensor.wait_ge(dma_sem, NDMA)
            for k in range(NT):
                W = (k + 1) * TW
                tensor.wait_ge(ve_sem, k + 1)
                for c in range(W // MMW):
                    mm = tensor.matmul(
                        ps[:, :], ones[:, 0:1],
                        wbufs[k % 2][:, c * MMW:(c + 1) * MMW],
                        start=(k == 0 and c == 0),
                        stop=(k == NT - 1 and c == (W // MMW) - 1),
                    )
                mm.then_inc(te_sem, 1)

    return nc


def _get_nc():
    global _NC_CACHE
    if _NC_CACHE is None:
        _NC_CACHE = build_nc()
    return _NC_CACHE


def _make_in_maps(pred, target):
    order = np.argsort(target, kind="stable")
    t = target[order].astype(np.float64)
    p = pred[order].astype(np.float64)

    tcneg = np.ascontiguousarray(
        np.broadcast_to((-t).astype(_bf16)[None, :], (P, B)))
    pcb2 = np.ascontiguousarray(
        np.broadcast_to((p / BETA).astype(_bf16)[None, :], (P, B)))
    ones = np.ones((P, 1), dtype=_bf16)

    jw = np.arange(TW)[None, :]
    pp = np.arange(P)[:, None]

    in_maps = []
    for c in range(NCORES):
        # rows[p, k] = global row of partition p in tile k
        rows = (8 * np.arange(NT)[None, :] + c) * P + pp
        tr = t[rows]
        in_maps.append({
            "tcneg": tcneg,
            "pcb2": pcb2,
            "mask": np.where(jw < c * P + pp, 30000.0, -30000.0).astype(_bf16),
            "trt": tr.astype(np.float32),
            "lnb": (1.0 + GAMMA * tr).astype(np.float32),
            "prb": (p[rows] / BETA).astype(np.float32),
            "ones": ones,
        })
    return in_maps


def kernel(pred, target):
    pred = np.asarray(pred, dtype=np.float32)
    target = np.asarray(target, dtype=np.float32)
    in_maps = _make_in_maps(pred, target)
    nc = _get_nc()
    res = run_bass_kernel_spmd(nc, in_maps, core_ids=list(range(NCORES)))
    total = 0.0
    for r in res.results:
        total += np.asarray(r["out"], dtype=np.float64).sum()
    K = B * (B - 1) // 2
    return np.float32(BETA * total / K)


# revision 8
# speedup vs baseline: 1.3904x; 1.3904x over previous
"""AdaptiveBoundaryRankingLoss on 8 TRN2 NeuronCores.

Math: loss = sum_{i<j} relu(boundary(|dt|) - (p_i-p_j)*sign(dt)) / K,
  dt = t_i - t_j, boundary(a) = BETA*a/(1+GAMMA*a), K = B(B-1)/2.

Host sorts (pred,target) by target ascending (the loss is a sum over
unordered pairs, so relabeling is free). After sorting, for i>j
(strict lower triangle) sign(t_i - t_j) = +1, so with
m(a) = a/(1+GAMMA*a), a = t_i - t_j >= 0, dp' = (p_i - p_j)/BETA:
  loss = BETA/K * sum_{i>j} relu(m(a) - dp').

m(a) is approximated per row by a minimax quadratic on a in [0, L_row]
(L_row = t_row - t_min; residual <= |m'''| L^3/192 ~ 3e-4 worst case):
  m(a) ~= beta_r - (s_r*(a - a0_r))^2.
The Square activation computes q = (scale*(-tc) + bias)^2 with
per-partition scale/bias APs, so one ScalarE pass yields q.
VectorE then computes u = -q + (beta_r - pr') (tensor_scalar),
z = u + pc' (tensor_tensor add) = m - dp', masks the triangular edge
(tensor_tensor min with a +/-30000 mask), and w = relu(z)
(tensor_scalar max). TensorE reduces: psum[8,512] += ones^T @ w.
Host: loss = BETA * sum(psum row) / K.

Work split: 64 row-blocks of 128 rows; core c takes blocks {8k+c},
tile k spans columns [0,(k+1)*1024) -> identical graph on all cores
(SPMD); per-core differences live in input data (tables + edge mask).
"""

import contextlib

import numpy as np
import ml_dtypes

import concourse.bass as bass
from concourse import mybir
from concourse.bass_utils import run_bass_kernel_spmd

B = 8192
BETA = 0.3
GAMMA = 0.1
NCORES = 8
NT = 8          # tiles per core
TW = 1024       # column granularity; tile k has W_k = (k+1)*TW columns
P = 128
MMW = 512       # matmul chunk width (one PSUM bank of f32)
MMREP = 8       # ones columns -> psum partitions (parallel psum writes)

_bf16 = ml_dtypes.bfloat16

_NC_CACHE = None


def build_nc():
    nc = bass.Bass(target_bir_lowering=False, debug=False)
    f32 = mybir.dt.float32
    bf16 = mybir.dt.bfloat16
    A = mybir.AluOpType

    tcneg_d = nc.declare_dram_parameter("tcneg", [P, B], bf16, isOutput=False)
    pcb2_d = nc.declare_dram_parameter("pcb2", [P, B], bf16, isOutput=False)
    mask_d = nc.declare_dram_parameter("mask", [P, TW], bf16, isOutput=False)
    sqs_d = nc.declare_dram_parameter("sqs", [P, NT], f32, isOutput=False)
    sqb_d = nc.declare_dram_parameter("sqb", [P, NT], f32, isOutput=False)
    ubt_d = nc.declare_dram_parameter("ubt", [P, NT], f32, isOutput=False)
    ones_d = nc.declare_dram_parameter("ones", [P, MMREP], bf16, isOutput=False)
    out_d = nc.declare_dram_parameter("out", [1, MMW], f32, isOutput=True)

    NDMA = 7 * 16

    es = contextlib.ExitStack()
    with es:
        def sb(name, shape, dtype):
            return es.enter_context(nc.sbuf_tensor(name, shape, dtype))

        tcneg = sb("tcneg_s", [P, B], bf16)
        pcb2 = sb("pcb2_s", [P, B], bf16)
        mask = sb("mask_s", [P, TW], bf16)
        sqs = sb("sqs_s", [P, NT], f32)
        sqb = sb("sqb_s", [P, NT], f32)
        ubt = sb("ubt_s", [P, NT], f32)
        ones = sb("ones_s", [P, MMREP], bf16)
        q0 = sb("q0", [P, B], bf16)
        q1 = sb("q1", [P, B], bf16)
        ub = sb("ub", [P, B], bf16)
        zb = sb("zb", [P, B], bf16)
        w0 = sb("w0", [P, B], bf16)
        w1 = sb("w1", [P, B], bf16)
        osb = sb("osb", [1, MMW], f32)
        ps = es.enter_context(nc.psum_tensor("ps", [MMREP, MMW], f32))
        dma_sem = es.enter_context(nc.semaphore("dma_sem"))
        se_sem = es.enter_context(nc.semaphore("se_sem"))
        ve_sem = es.enter_context(nc.semaphore("ve_sem"))
        te_sem = es.enter_context(nc.semaphore("te_sem"))
        block = es.enter_context(nc.Block())

        qbufs = [q0, q1]
        wbufs = [w0, w1]

        @block.sync
        def _(sync):
            for dst, src in [
                (tcneg, tcneg_d), (pcb2, pcb2_d), (mask, mask_d),
                (sqs, sqs_d), (sqb, sqb_d), (ubt, ubt_d), (ones, ones_d),
            ]:
                sync.dma_start(out=dst[:, :], in_=src[:, :]).then_inc(dma_sem, 16)
            sync.wait_ge(ve_sem, NT + 1)
            sync.dma_start(out=out_d[:, :], in_=osb[:, :]).then_inc(dma_sem, 16)

        @block.scalar
        def _(scalar):
            scalar.wait_ge(dma_sem, NDMA)
            for k in range(NT):
                W = (k + 1) * TW
                if k >= 2:
                    scalar.wait_ge(ve_sem, k - 1)
                # q = (s_r*(a - a0_r))^2 ; in = -tc, scale/bias per-row APs
                scalar.activation(
                    qbufs[k % 2][:, :W], tcneg[:, :W],
                    mybir.ActivationFunctionType.Square,
                    bias=sqb[:, k:k + 1], scale=sqs[:, k:k + 1],
                ).then_inc(se_sem, 1)

        @block.vector
        def _(vector):
            vector.wait_ge(dma_sem, NDMA)
            for k in range(NT):
                W = (k + 1) * TW
                vector.wait_ge(se_sem, k + 1)
                # u = -q + (beta_r - pr')
                vector.tensor_scalar(
                    out=ub[:, :W], in0=qbufs[k % 2][:, :W],
                    scalar1=-1.0, scalar2=ubt[:, k:k + 1],
                    op0=A.mult, op1=A.add,
                )
                # z = u + pc' = m - dp'
                vector.tensor_tensor(
                    out=zb[:, :W], in0=ub[:, :W], in1=pcb2[:, :W], op=A.add,
                )
                # edge window: invalid (j >= i) elements -> -30000
                vector.tensor_tensor(
                    out=zb[:, k * TW:W], in0=zb[:, k * TW:W], in1=mask[:, :],
                    op=A.min,
                )
                if k >= 2:
                    vector.wait_ge(te_sem, k - 1)
                # w = relu(z)
                vector.tensor_scalar(
                    out=wbufs[k % 2][:, :W], in0=zb[:, :W],
                    scalar1=0.0, scalar2=None, op0=A.max,
                ).then_inc(ve_sem, 1)
            vector.wait_ge(te_sem, NT)
            vector.tensor_copy(osb[:, :], ps[0:1, :]).then_inc(ve_sem, 1)

        @block.tensor
        def _(tensor):
            tensor.wait_ge(dma_sem, NDMA)
            for k in range(NT):
                W = (k + 1) * TW
                tensor.wait_ge(ve_sem, k + 1)
                for c in range(W // MMW):
                    mm = tensor.matmul(
                        ps[:, :], ones[:, :],
                        wbufs[k % 2][:, c * MMW:(c + 1) * MMW],
                        start=(k == 0 and c == 0),
                        stop=(k == NT - 1 and c == (W // MMW) - 1),
                    )
                mm.then_inc(te_sem, 1)

    return nc


def _get_nc():
    global _NC_CACHE
    if _NC_CACHE is None:
        _NC_CACHE = build_nc()
    return _NC_CACHE


def _quad_fit_rows(L, n=48):
    """Vectorized per-row quadratic fit of m(a)=a/(1+G*a) on [0, L_r]
    via Chebyshev interpolation (degree 2). Returns coeff arrays
    (c0, c1, c2) of p(a) = c0 + c1*a + c2*a^2."""
    L = np.maximum(np.asarray(L, np.float64), 1e-3)
    n_ = n
    xk = np.cos((2 * np.arange(n_) + 1) * np.pi / (2 * n_))
    a = (xk[None, :] + 1.0) * 0.5 * L[:, None]          # [rows, n]
    f = a / (1.0 + GAMMA * a)
    b0 = f @ (np.ones_like(xk) / n_)
    b1 = f @ (xk * 2.0 / n_)
    b2 = f @ ((2 * xk * xk - 1.0) * 2.0 / n_)
    # p(x) = (b0 - b2) + b1*x + 2*b2*x^2,  x = 2a/L - 1
    A0 = b0 - b2
    A1 = b1
    A2 = 2 * b2
    c0 = A0 - A1 + A2
    c1 = (A1 - 2 * A2) * 2.0 / L
    c2 = A2 * 4.0 / (L * L)
    return c0, c1, c2


def _make_in_maps(pred, target):
    order = np.argsort(target, kind="stable")
    t = target[order].astype(np.float64)
    p = pred[order].astype(np.float64)
    tmin = t[0]

    tcneg = np.ascontiguousarray(
        np.broadcast_to((-t).astype(_bf16)[None, :], (P, B)))
    pcb2 = np.ascontiguousarray(
        np.broadcast_to((p / BETA).astype(_bf16)[None, :], (P, B)))
    ones = np.ones((P, MMREP), dtype=_bf16)

    jw = np.arange(TW)[None, :]
    pp = np.arange(P)[:, None]

    in_maps = []
    for c in range(NCORES):
        # rows[p, k] = global row of partition p in tile k
        rows = (8 * np.arange(NT)[None, :] + c) * P + pp
        tr = t[rows]                       # [128, 8]
        pr = p[rows] / BETA
        c0, c1, c2 = _quad_fit_rows((tr - tmin).ravel())
        c0 = c0.reshape(P, NT)
        c1 = c1.reshape(P, NT)
        c2 = np.minimum(c2.reshape(P, NT), -1e-8)
        s = np.sqrt(-c2)
        a0 = -c1 / (2 * c2)
        beta_r = c0 - c2 * a0 * a0
        # q = (s*(a - a0))^2 with a = tr - tc: in = -tc -> scale = s,
        # bias = s*(tr - a0)
        in_maps.append({
            "tcneg": tcneg,
            "pcb2": pcb2,
            "mask": np.where(jw < c * P + pp, 30000.0, -30000.0).astype(_bf16),
            "sqs": s.astype(np.float32),
            "sqb": (s * (tr - a0)).astype(np.float32),
            "ubt": (beta_r - pr).astype(np.float32),
            "ones": ones,
        })
    return in_maps


def kernel(pred, target):
    pred = np.asarray(pred, dtype=np.float32)
    target = np.asarray(target, dtype=np.float32)
    in_maps = _make_in_maps(pred, target)
    nc = _get_nc()
    res = run_bass_kernel_spmd(nc, in_maps, core_ids=list(range(NCORES)))
    total = 0.0
    for r in res.results:
        total += np.asarray(r["out"], dtype=np.float64).sum()
    K = B * (B - 1) // 2
    return np.float32(BETA * total / K)


# revision 9
# speedup vs baseline: 1.7361x; 1.2486x over previous
"""AdaptiveBoundaryRankingLoss on 8 TRN2 NeuronCores.

Math: loss = sum_{i<j} relu(boundary(|dt|) - (p_i-p_j)*sign(dt)) / K,
  dt = t_i - t_j, boundary(a) = BETA*a/(1+GAMMA*a), K = B(B-1)/2.

Host sorts (pred,target) by target ascending (the loss is a sum over
unordered pairs, so relabeling is free). After sorting, for i>j
(strict lower triangle) sign(t_i - t_j) = +1, so with
m(a) = a/(1+GAMMA*a), a = t_i - t_j >= 0, dp' = (p_i - p_j)/BETA:
  loss = BETA/K * sum_{i>j} relu(m(a) - dp').

m(a) is approximated per row by a minimax quadratic on a in [0, L_row]
(L_row = t_row - t_min; residual <= |m'''| L^3/192 ~ 3e-4 worst case):
  m(a) ~= beta_r - (s_r*(a - a0_r))^2.

Per 128-row tile k (columns [0, W), W = (k+1)*1024):
  ScalarE: q = Square(sqs_r*(-tc) + sqb_r)        (= (s_r*(a-a0_r))^2)
  VectorE: z = pc' - q        (tensor_tensor sub; edge window reads a
           host-baked edge_pc with invalid columns = -30000)
           w = relu(z + (beta_r - pr'))   (one 2-op tensor_scalar)
  TensorE: psum[1,512] += ones[128,1]^T @ w[:,chunk]  (column sums)
Host: loss = BETA * sum(psum) / K.

Work split: 64 row-blocks of 128 rows; core c takes blocks {8k+c},
tile k spans columns [0,(k+1)*1024) -> identical graph on all cores
(SPMD); per-core differences live in input data (tables + edge_pc).
"""

import contextlib

import numpy as np
import ml_dtypes

import concourse.bass as bass
from concourse import mybir
from concourse.bass_utils import run_bass_kernel_spmd

B = 8192
BETA = 0.3
GAMMA = 0.1
NCORES = 8
NT = 8          # tiles per core
TW = 1024       # column granularity; tile k has W_k = (k+1)*TW columns
P = 128
MMW = 512       # matmul chunk width (one PSUM bank of f32)
DMA_SPLIT = 2 * TW   # early columns of tcneg/pcb2 (tiles 0-1 start sooner)

_bf16 = ml_dtypes.bfloat16

_NC_CACHE = None


def build_nc():
    nc = bass.Bass(target_bir_lowering=False, debug=False)
    f32 = mybir.dt.float32
    bf16 = mybir.dt.bfloat16
    A = mybir.AluOpType

    tcneg_d = nc.declare_dram_parameter("tcneg", [P, B], bf16, isOutput=False)
    pcb2_d = nc.declare_dram_parameter("pcb2", [P, B], bf16, isOutput=False)
    edge_d = nc.declare_dram_parameter("edgepc", [P, NT * TW], bf16, isOutput=False)
    sqs_d = nc.declare_dram_parameter("sqs", [P, NT], f32, isOutput=False)
    sqb_d = nc.declare_dram_parameter("sqb", [P, NT], f32, isOutput=False)
    ubt_d = nc.declare_dram_parameter("ubt", [P, NT], f32, isOutput=False)
    ones_d = nc.declare_dram_parameter("ones", [P, 1], bf16, isOutput=False)
    out_d = nc.declare_dram_parameter("out", [1, MMW], f32, isOutput=True)

    # DMA order: 4 small tables, first DMA_SPLIT cols of tcneg/pcb2,
    # edgepc, then the remaining cols.  Sem thresholds:
    SEM_TABLES = 4 * 16          # sqs, sqb, ubt, ones
    SEM_EARLY = SEM_TABLES + 3 * 16   # + tcneg[:S], pcb2[:S], edgepc
    SEM_ALL = SEM_EARLY + 2 * 16      # + tcneg[S:], pcb2[S:]

    es = contextlib.ExitStack()
    with es:
        def sb(name, shape, dtype):
            return es.enter_context(nc.sbuf_tensor(name, shape, dtype))

        tcneg = sb("tcneg_s", [P, B], bf16)
        pcb2 = sb("pcb2_s", [P, B], bf16)
        edgepc = sb("edgepc_s", [P, NT * TW], bf16)
        sqs = sb("sqs_s", [P, NT], f32)
        sqb = sb("sqb_s", [P, NT], f32)
        ubt = sb("ubt_s", [P, NT], f32)
        ones = sb("ones_s", [P, 1], bf16)
        q0 = sb("q0", [P, B], bf16)
        q1 = sb("q1", [P, B], bf16)
        q2 = sb("q2", [P, B], bf16)
        zb = sb("zb", [P, B], bf16)
        w0 = sb("w0", [P, B], bf16)
        w1 = sb("w1", [P, B], bf16)
        osb = sb("osb", [1, MMW], f32)
        ps = es.enter_context(nc.psum_tensor("ps", [1, MMW], f32))
        dma_sem = es.enter_context(nc.semaphore("dma_sem"))
        se_sem = es.enter_context(nc.semaphore("se_sem"))
        ve_sem = es.enter_context(nc.semaphore("ve_sem"))
        te_sem = es.enter_context(nc.semaphore("te_sem"))
        block = es.enter_context(nc.Block())

        qbufs = [q0, q1, q2]
        wbufs = [w0, w1]
        S = DMA_SPLIT

        @block.sync
        def _(sync):
            for dst, src in [
                (sqs[:, :], sqs_d[:, :]), (sqb[:, :], sqb_d[:, :]),
                (ubt[:, :], ubt_d[:, :]), (ones[:, :], ones_d[:, :]),
                (tcneg[:, :S], tcneg_d[:, :S]), (pcb2[:, :S], pcb2_d[:, :S]),
                (edgepc[:, :], edge_d[:, :]),
                (tcneg[:, S:], tcneg_d[:, S:]), (pcb2[:, S:], pcb2_d[:, S:]),
            ]:
                sync.dma_start(out=dst, in_=src).then_inc(dma_sem, 16)
            sync.wait_ge(ve_sem, NT + 1)
            sync.dma_start(out=out_d[:, :], in_=osb[:, :]).then_inc(dma_sem, 16)

        @block.scalar
        def _(scalar):
            scalar.wait_ge(dma_sem, SEM_EARLY)
            for k in range(NT):
                W = (k + 1) * TW
                if k == 2:
                    scalar.wait_ge(dma_sem, SEM_ALL)
                if k >= 3:
                    scalar.wait_ge(ve_sem, k - 2)
                # q = (s_r*(a - a0_r))^2 ; in = -tc, scale/bias per-row APs
                scalar.activation(
                    qbufs[k % 3][:, :W], tcneg[:, :W],
                    mybir.ActivationFunctionType.Square,
                    bias=sqb[:, k:k + 1], scale=sqs[:, k:k + 1],
                ).then_inc(se_sem, 1)

        @block.vector
        def _(vector):
            vector.wait_ge(dma_sem, SEM_EARLY)
            for k in range(NT):
                W = (k + 1) * TW
                if k == 2:
                    vector.wait_ge(dma_sem, SEM_ALL)
                vector.wait_ge(se_sem, k + 1)
                q = qbufs[k % 3]
                # z = pc' - q  (main region), edge window uses baked edge_pc
                if k > 0:
                    vector.tensor_tensor(
                        out=zb[:, :k * TW], in0=pcb2[:, :k * TW],
                        in1=q[:, :k * TW], op=A.subtract,
                    )
                vector.tensor_tensor(
                    out=zb[:, k * TW:W], in0=edgepc[:, k * TW:W],
                    in1=q[:, k * TW:W], op=A.subtract,
                )
                if k >= 2:
                    vector.wait_ge(te_sem, k - 1)
                # w = relu(z + (beta_r - pr'))
                vector.tensor_scalar(
                    out=wbufs[k % 2][:, :W], in0=zb[:, :W],
                    scalar1=ubt[:, k:k + 1], scalar2=0.0,
                    op0=A.add, op1=A.max,
                ).then_inc(ve_sem, 1)
            vector.wait_ge(te_sem, NT)
            vector.tensor_copy(osb[:, :], ps[0:1, :]).then_inc(ve_sem, 1)

        @block.tensor
        def _(tensor):
            tensor.wait_ge(dma_sem, SEM_TABLES)
            for k in range(NT):
                W = (k + 1) * TW
                tensor.wait_ge(ve_sem, k + 1)
                for c in range(W // MMW):
                    mm = tensor.matmul(
                        ps[:, :], ones[:, :],
                        wbufs[k % 2][:, c * MMW:(c + 1) * MMW],
                        start=(k == 0 and c == 0),
                        stop=(k == NT - 1 and c == (W // MMW) - 1),
                    )
                mm.then_inc(te_sem, 1)

    return nc


def _get_nc():
    global _NC_CACHE
    if _NC_CACHE is None:
        _NC_CACHE = build_nc()
    return _NC_CACHE


def _quad_fit_rows(L, n=48):
    """Vectorized per-row quadratic fit of m(a)=a/(1+G*a) on [0, L_r]
    via Chebyshev interpolation (degree 2). Returns coeff arrays
    (c0, c1, c2) of p(a) = c0 + c1*a + c2*a^2."""
    L = np.maximum(np.asarray(L, np.float64), 1e-3)
    n_ = n
    xk = np.cos((2 * np.arange(n_) + 1) * np.pi / (2 * n_))
    a = (xk[None, :] + 1.0) * 0.5 * L[:, None]          # [rows, n]
    f = a / (1.0 + GAMMA * a)
    b0 = f @ (np.ones_like(xk) / n_)
    b1 = f @ (xk * 2.0 / n_)
    b2 = f @ ((2 * xk * xk - 1.0) * 2.0 / n_)
    # p(x) = (b0 - b2) + b1*x + 2*b2*x^2,  x = 2a/L - 1
    A0 = b0 - b2
    A1 = b1
    A2 = 2 * b2
    c0 = A0 - A1 + A2
    c1 = (A1 - 2 * A2) * 2.0 / L
    c2 = A2 * 4.0 / (L * L)
    return c0, c1, c2


def _make_in_maps(pred, target):
    order = np.argsort(target, kind="stable")
    t = target[order].astype(np.float64)
    p = pred[order].astype(np.float64)
    tmin = t[0]

    tcneg_1d = (-t).astype(_bf16)
    pcb2_1d = (p / BETA).astype(_bf16)
    tcneg = np.ascontiguousarray(np.broadcast_to(tcneg_1d[None, :], (P, B)))
    pcb2 = np.ascontiguousarray(np.broadcast_to(pcb2_1d[None, :], (P, B)))
    ones = np.ones((P, 1), dtype=_bf16)

    jw = np.arange(TW)[None, :]
    pp = np.arange(P)[:, None]

    in_maps = []
    for c in range(NCORES):
        # rows[p, k] = global row of partition p in tile k
        rows = (8 * np.arange(NT)[None, :] + c) * P + pp
        tr = t[rows]                       # [128, 8]
        pr = p[rows] / BETA
        c0, c1, c2 = _quad_fit_rows((tr - tmin).ravel())
        c0 = c0.reshape(P, NT)
        c1 = c1.reshape(P, NT)
        c2 = np.minimum(c2.reshape(P, NT), -1e-8)
        s = np.sqrt(-c2)
        a0 = -c1 / (2 * c2)
        beta_r = c0 - c2 * a0 * a0
        # edge_pc[p, k*TW + jw] = valid ? pcb2[k*TW+jw] : -30000
        valid = (jw < c * P + pp)          # [128, TW], same for every k
        edge = np.empty((P, NT * TW), dtype=_bf16)
        for k in range(NT):
            vals = np.broadcast_to(pcb2_1d[k * TW:(k + 1) * TW][None, :], (P, TW))
            edge[:, k * TW:(k + 1) * TW] = np.where(
                valid, vals, _bf16(-30000.0))
        in_maps.append({
            "tcneg": tcneg,
            "pcb2": pcb2,
            "edgepc": edge,
            "sqs": s.astype(np.float32),
            "sqb": (s * (tr - a0)).astype(np.float32),
            "ubt": (beta_r - pr).astype(np.float32),
            "ones": ones,
        })
    return in_maps


def kernel(pred, target):
    pred = np.asarray(pred, dtype=np.float32)
    target = np.asarray(target, dtype=np.float32)
    in_maps = _make_in_maps(pred, target)
    nc = _get_nc()
    res = run_bass_kernel_spmd(nc, in_maps, core_ids=list(range(NCORES)))
    total = 0.0
    for r in res.results:
        total += np.asarray(r["out"], dtype=np.float64).sum()
    K = B * (B - 1) // 2
    return np.float32(BETA * total / K)


# revision 10
# speedup vs baseline: 1.9182x; 1.1049x over previous
"""AdaptiveBoundaryRankingLoss on 8 TRN2 NeuronCores.

Math: loss = sum_{i<j} relu(boundary(|dt|) - (p_i-p_j)*sign(dt)) / K,
  dt = t_i - t_j, boundary(a) = BETA*a/(1+GAMMA*a), K = B(B-1)/2.

Host sorts (pred,target) by target ascending (the loss is a sum over
unordered pairs, so relabeling is free). After sorting, for i>j
(strict lower triangle) sign(t_i - t_j) = +1, so with
m(a) = a/(1+GAMMA*a), a = t_i - t_j >= 0, dp' = (p_i - p_j)/BETA:
  loss = BETA/K * sum_{i>j} relu(m(a) - dp').

m(a) is approximated per row by a minimax quadratic on a in [0, L_row]
(L_row = t_row - t_min; residual <= |m'''| L^3/192 ~ 3e-4 worst case):
  m(a) ~= beta_r - (s_r*(a - a0_r))^2,  q := (s_r*(a - a0_r))^2.

Per 128-row tile k (columns [0, W), W = (k+1)*1024):
  q: tiles 0-2 on VectorE (x = sqs*(-tc)+sqb via 2-op tensor_scalar,
     q = x*x via tensor_tensor); tiles 3-7 on ScalarE (one Square
     activation with per-row scale/bias APs).
  z = pc' - q   (tensor_tensor sub; the 1024-wide diagonal edge window
     reads a host-baked edge_pc whose invalid columns are -30000)
  w = relu(z + (beta_r - pr'))   (one 2-op tensor_scalar)
  TensorE: psum[1,512] += ones[128,1]^T @ w[:,chunk]  (column sums)
Host: loss = BETA * sum(psum) / K.

DMA is split across two queues with independent semaphores: sync
carries ScalarE-side inputs (tables + tcneg in 3 column chunks),
gpsimd carries VectorE-side inputs (pcb2/edge_pc chunks), so each
engine gates only on the bytes it actually reads and tile 0 starts
after ~1MB instead of 6MB. Tile 7 is split into two column phases so
TensorE's final matmuls start before the last relu finishes.

Work split: 64 row-blocks of 128 rows; core c takes blocks {8k+c},
tile k spans columns [0,(k+1)*1024) -> identical graph on all cores
(SPMD); per-core differences live in input data (tables + edge_pc).
"""

import contextlib

import numpy as np
import ml_dtypes

import concourse.bass as bass
from concourse import mybir
from concourse.bass_utils import run_bass_kernel_spmd

B = 8192
BETA = 0.3
GAMMA = 0.1
NCORES = 8
NT = 8          # tiles per core
TW = 1024       # column granularity; tile k has W_k = (k+1)*TW columns
P = 128
MMW = 512       # matmul chunk width (one PSUM bank of f32)
NVQ = 3         # tiles 0..NVQ-1 compute q on VectorE instead of ScalarE
TAIL = 2 * TW   # tile 7 phase-2 width (TensorE tail shortening)

# cumulative relu-instruction count after tile k (tile 7 has 2 phases)
_R = [1, 2, 3, 4, 5, 6, 7, 9]

_bf16 = ml_dtypes.bfloat16

_NC_CACHE = None


def build_nc():
    nc = bass.Bass(target_bir_lowering=False, debug=False)
    f32 = mybir.dt.float32
    bf16 = mybir.dt.bfloat16
    A = mybir.AluOpType

    tcneg_d = nc.declare_dram_parameter("tcneg", [P, B], bf16, isOutput=False)
    pcb2_d = nc.declare_dram_parameter("pcb2", [P, B], bf16, isOutput=False)
    edge_d = nc.declare_dram_parameter("edgepc", [P, NT * TW], bf16, isOutput=False)
    sqs_d = nc.declare_dram_parameter("sqs", [P, NT], f32, isOutput=False)
    sqb_d = nc.declare_dram_parameter("sqb", [P, NT], f32, isOutput=False)
    ubt_d = nc.declare_dram_parameter("ubt", [P, NT], f32, isOutput=False)
    ones_d = nc.declare_dram_parameter("ones", [P, 1], bf16, isOutput=False)
    out_d = nc.declare_dram_parameter("out", [1, MMW], f32, isOutput=True)

    es = contextlib.ExitStack()
    with es:
        def sb(name, shape, dtype):
            return es.enter_context(nc.sbuf_tensor(name, shape, dtype))

        tcneg = sb("tcneg_s", [P, B], bf16)
        pcb2 = sb("pcb2_s", [P, B], bf16)
        edgepc = sb("edgepc_s", [P, NT * TW], bf16)
        sqs = sb("sqs_s", [P, NT], f32)
        sqb = sb("sqb_s", [P, NT], f32)
        ubt = sb("ubt_s", [P, NT], f32)
        ones = sb("ones_s", [P, 1], bf16)
        q0 = sb("q0", [P, B], bf16)
        q1 = sb("q1", [P, B], bf16)
        q2 = sb("q2", [P, B], bf16)
        xb = sb("xb", [P, NVQ * TW], bf16)
        qv = sb("qv", [P, NVQ * TW], bf16)
        zb = sb("zb", [P, B], bf16)
        w0 = sb("w0", [P, B], bf16)
        w1 = sb("w1", [P, B], bf16)
        osb = sb("osb", [1, MMW], f32)
        ps = es.enter_context(nc.psum_tensor("ps", [1, MMW], f32))
        dma_a = es.enter_context(nc.semaphore("dma_a"))
        dma_b = es.enter_context(nc.semaphore("dma_b"))
        se_sem = es.enter_context(nc.semaphore("se_sem"))
        ve_sem = es.enter_context(nc.semaphore("ve_sem"))
        te_sem = es.enter_context(nc.semaphore("te_sem"))
        block = es.enter_context(nc.Block())

        qbufs = [q0, q1, q2]
        wbufs = [w0, w1]

        @block.sync
        def _(sync):
            # queue a: ScalarE-side bytes (+ ones for TensorE)
            for dst, src in [
                (sqs[:, :], sqs_d[:, :]),                       # a>=16
                (sqb[:, :], sqb_d[:, :]),                       # a>=32
                (ones[:, :], ones_d[:, :]),                     # a>=48
                (tcneg[:, :TW], tcneg_d[:, :TW]),               # a>=64
                (tcneg[:, TW:4 * TW], tcneg_d[:, TW:4 * TW]),   # a>=80
                (tcneg[:, 4 * TW:], tcneg_d[:, 4 * TW:]),       # a>=96
            ]:
                sync.dma_start(out=dst, in_=src).then_inc(dma_a, 16)
            sync.wait_ge(ve_sem, _R[NT - 1] + 1)
            sync.dma_start(out=out_d[:, :], in_=osb[:, :]).then_inc(dma_a, 16)

        @block.gpsimd
        def _(gpsimd):
            # queue b: VectorE-side bytes
            for dst, src in [
                (ubt[:, :], ubt_d[:, :]),                               # b>=16
                (pcb2[:, :TW], pcb2_d[:, :TW]),                         # b>=32
                (edgepc[:, :TW], edge_d[:, :TW]),                       # b>=48
                (pcb2[:, TW:4 * TW], pcb2_d[:, TW:4 * TW]),             # b>=64
                (edgepc[:, TW:4 * TW], edge_d[:, TW:4 * TW]),           # b>=80
                (pcb2[:, 4 * TW:], pcb2_d[:, 4 * TW:]),                 # b>=96
                (edgepc[:, 4 * TW:], edge_d[:, 4 * TW:]),               # b>=112
            ]:
                gpsimd.dma_start(out=dst, in_=src).then_inc(dma_b, 16)

        @block.scalar
        def _(scalar):
            # dummy 1-element Square: pulls ACT_TABLE_LOAD to t=0
            scalar.activation(
                q0[:, 0:1], q0[:, 0:1],
                mybir.ActivationFunctionType.Square,
            )
            scalar.wait_ge(dma_a, 80)
            for k in range(NVQ, NT):
                W = (k + 1) * TW
                if k == 4:
                    scalar.wait_ge(dma_a, 96)
                if k >= NVQ + 3:
                    # q buffer reuse: wait until VE consumed tile k-3
                    scalar.wait_ge(ve_sem, _R[k - 3])
                # q = (s_r*(a - a0_r))^2 ; in = -tc, scale/bias per-row APs
                scalar.activation(
                    qbufs[k % 3][:, :W], tcneg[:, :W],
                    mybir.ActivationFunctionType.Square,
                    bias=sqb[:, k:k + 1], scale=sqs[:, k:k + 1],
                ).then_inc(se_sem, 1)

        @block.vector
        def _(vector):
            vector.wait_ge(dma_a, 64)
            vector.wait_ge(dma_b, 48)
            for k in range(NT):
                W = (k + 1) * TW
                if k == 1:
                    vector.wait_ge(dma_a, 80)
                    vector.wait_ge(dma_b, 80)
                if k == 4:
                    vector.wait_ge(dma_b, 112)
                if k < NVQ:
                    # q on VectorE: x = sqs*(-tc) + sqb ; q = x*x
                    vector.tensor_scalar(
                        out=xb[:, :W], in0=tcneg[:, :W],
                        scalar1=sqs[:, k:k + 1], scalar2=sqb[:, k:k + 1],
                        op0=A.mult, op1=A.add,
                    )
                    vector.tensor_tensor(
                        out=qv[:, :W], in0=xb[:, :W], in1=xb[:, :W], op=A.mult,
                    )
                    q = qv
                else:
                    vector.wait_ge(se_sem, k - NVQ + 1)
                    q = qbufs[k % 3]
                phases = [(0, W)] if k < NT - 1 else [(0, W - TAIL), (W - TAIL, W)]
                for lo, hi in phases:
                    # z = pc' - q (main), edge window uses baked edge_pc
                    mhi = min(hi, k * TW)
                    if mhi > lo:
                        vector.tensor_tensor(
                            out=zb[:, lo:mhi], in0=pcb2[:, lo:mhi],
                            in1=q[:, lo:mhi], op=A.subtract,
                        )
                    elo = max(lo, k * TW)
                    if hi > elo:
                        vector.tensor_tensor(
                            out=zb[:, elo:hi], in0=edgepc[:, elo:hi],
                            in1=q[:, elo:hi], op=A.subtract,
                        )
                    if k >= 2 and lo == 0:
                        vector.wait_ge(te_sem, k - 1)
                    # w = relu(z + (beta_r - pr'))
                    vector.tensor_scalar(
                        out=wbufs[k % 2][:, lo:hi], in0=zb[:, lo:hi],
                        scalar1=ubt[:, k:k + 1], scalar2=0.0,
                        op0=A.add, op1=A.max,
                    ).then_inc(ve_sem, 1)
            vector.wait_ge(te_sem, NT)
            vector.tensor_copy(osb[:, :], ps[0:1, :]).then_inc(ve_sem, 1)

        @block.tensor
        def _(tensor):
            tensor.wait_ge(dma_a, 48)
            for k in range(NT):
                W = (k + 1) * TW
                phase1_end = W if k < NT - 1 else W - TAIL
                tensor.wait_ge(ve_sem, _R[k - 1] + 1 if k > 0 else 1)
                for c in range(W // MMW):
                    if c * MMW >= phase1_end:
                        tensor.wait_ge(ve_sem, _R[k])
                    mm = tensor.matmul(
                        ps[:, :], ones[:, :],
                        wbufs[k % 2][:, c * MMW:(c + 1) * MMW],
                        start=(k == 0 and c == 0),
                        stop=(k == NT - 1 and c == (W // MMW) - 1),
                    )
                mm.then_inc(te_sem, 1)

    return nc


def _get_nc():
    global _NC_CACHE
    if _NC_CACHE is None:
        _NC_CACHE = build_nc()
    return _NC_CACHE


def _quad_fit_rows(L, n=48):
    """Vectorized per-row quadratic fit of m(a)=a/(1+G*a) on [0, L_r]
    via Chebyshev interpolation (degree 2). Returns coeff arrays
    (c0, c1, c2) of p(a) = c0 + c1*a + c2*a^2."""
    L = np.maximum(np.asarray(L, np.float64), 1e-3)
    n_ = n
    xk = np.cos((2 * np.arange(n_) + 1) * np.pi / (2 * n_))
    a = (xk[None, :] + 1.0) * 0.5 * L[:, None]          # [rows, n]
    f = a / (1.0 + GAMMA * a)
    b0 = f @ (np.ones_like(xk) / n_)
    b1 = f @ (xk * 2.0 / n_)
    b2 = f @ ((2 * xk * xk - 1.0) * 2.0 / n_)
    # p(x) = (b0 - b2) + b1*x + 2*b2*x^2,  x = 2a/L - 1
    A0 = b0 - b2
    A1 = b1
    A2 = 2 * b2
    c0 = A0 - A1 + A2
    c1 = (A1 - 2 * A2) * 2.0 / L
    c2 = A2 * 4.0 / (L * L)
    return c0, c1, c2


def _make_in_maps(pred, target):
    order = np.argsort(target, kind="stable")
    t = target[order].astype(np.float64)
    p = pred[order].astype(np.float64)
    tmin = t[0]

    tcneg_1d = (-t).astype(_bf16)
    pcb2_1d = (p / BETA).astype(_bf16)
    tcneg = np.ascontiguousarray(np.broadcast_to(tcneg_1d[None, :], (P, B)))
    pcb2 = np.ascontiguousarray(np.broadcast_to(pcb2_1d[None, :], (P, B)))
    ones = np.ones((P, 1), dtype=_bf16)

    jw = np.arange(TW)[None, :]
    pp = np.arange(P)[:, None]

    in_maps = []
    for c in range(NCORES):
        # rows[p, k] = global row of partition p in tile k
        rows = (8 * np.arange(NT)[None, :] + c) * P + pp
        tr = t[rows]                       # [128, 8]
        pr = p[rows] / BETA
        c0, c1, c2 = _quad_fit_rows((tr - tmin).ravel())
        c0 = c0.reshape(P, NT)
        c1 = c1.reshape(P, NT)
        c2 = np.minimum(c2.reshape(P, NT), -1e-8)
        s = np.sqrt(-c2)
        a0 = -c1 / (2 * c2)
        beta_r = c0 - c2 * a0 * a0
        # edge_pc[p, k*TW + jw] = valid ? pcb2[k*TW+jw] : -30000
        valid = (jw < c * P + pp)          # [128, TW], same for every k
        edge = np.empty((P, NT * TW), dtype=_bf16)
        for k in range(NT):
            vals = np.broadcast_to(pcb2_1d[k * TW:(k + 1) * TW][None, :], (P, TW))
            edge[:, k * TW:(k + 1) * TW] = np.where(
                valid, vals, _bf16(-30000.0))
        in_maps.append({
            "tcneg": tcneg,
            "pcb2": pcb2,
            "edgepc": edge,
            "sqs": s.astype(np.float32),
            "sqb": (s * (tr - a0)).astype(np.float32),
            "ubt": (beta_r - pr).astype(np.float32),
            "ones": ones,
        })
    return in_maps


def kernel(pred, target):
    pred = np.asarray(pred, dtype=np.float32)
    target = np.asarray(target, dtype=np.float32)
    in_maps = _make_in_maps(pred, target)
    nc = _get_nc()
    res = run_bass_kernel_spmd(nc, in_maps, core_ids=list(range(NCORES)))
    total = 0.0
    for r in res.results:
        total += np.asarray(r["out"], dtype=np.float64).sum()
    K = B * (B - 1) // 2
    return np.float32(BETA * total / K)
